# revision 38
# baseline (speedup 1.0000x reference)
"""Trainium2 Bass kernel for nn_BiasedMultiHeadAttention (B=4, H=16, L=1024, E=1024).

Sharding: 64 (batch, head) pairs over 8 cores -> core c handles batch b=c//2,
heads h0=(c%2)*8 .. h0+8. Each core runs LayerNorm + its Q/K/V projection
slices + biased masked attention for its 8 heads + its slice of the output
projection (row-parallel). The two cores sharing a batch each return a partial
[L, E] out-projection (bf16); the host sums the pair, applies the query mask,
and adds residual + bo.

Host-side folding (exact algebra, done in fp32):
  - gamma/beta folded into the projection weights/biases
  - 1/sqrt(D) folded into Wq/bq
  - gate*bias pre-exponentiated: device computes exp(Q K^T) * egb where
    egb = max(exp(gate*bias) * keymask, 1e-30) (softmax shift/scale cancels in
    the normalization; key masking becomes a multiply-by-~zero; the 1e-30
    floor keeps denominators nonzero so no NaN guard is needed on device).
  - the query mask is applied on the host after gathering partials.

Device layouts (per core): attention runs transposed, logitsT[k, q], so the
softmax denominator falls out of the attention*V matmul via an appended
ones-column on V. Attention is a 3-stage head pipeline: QK/exp/egb-mul for
head h runs while the AV matmuls for head h-1 stream from fully staged
attention tiles (so the PE never stalls on the exp chain and HAM stays at
2.4 GHz), and head h-2's normalization (reciprocal-denominator broadcast via
rank-1 matmuls) trails behind.
"""
import numpy as np
import ml_dtypes
from contextlib import ExitStack

import concourse.bass as bass
import concourse.bacc as bacc
import concourse.tile as tile
from concourse import mybir
from concourse.bass_utils import run_bass_kernel_spmd

BF16 = mybir.dt.bfloat16
F32 = mybir.dt.float32
NBF16 = ml_dtypes.bfloat16
AF = mybir.ActivationFunctionType
ALU = mybir.AluOpType

P = 128
B, L, E, D, H = 4, 1024, 1024, 64, 16
HPC = 8            # heads per core
FL = HPC * D       # local feature width = 512
FC = FL // P       # 4 feature chunks
EC = E // P        # 8 embed chunks
LC = L // P        # 8 sequence chunks
NCORES = 8
LN_EPS = 1e-5

_NC = None


def _emit(nc, tc, ctx, xd, wq_d, wk_d, wv_d, wo_d, bq_d, bk_d, bv_d, eg_d, id_d,
          sel_d, out_d):
    sync = nc.sync
    x_t = xd.ap().rearrange("(t p) e -> t p e", p=P)
    out_t = out_d.ap().rearrange("(t p) e -> t p e", p=P)

    consts = ctx.enter_context(tc.tile_pool(name="consts", bufs=1))
    # egb prefetch pool lives at outer scope so its SBUF cannot alias the
    # projection-phase weight pool: the 16 MB of egb streams during the
    # projection phase instead of stalling the attention loop.
    egbp = ctx.enter_context(tc.tile_pool(name="egb", bufs=20))

    # x tiles first: the LayerNorm -> transpose -> projection critical path
    # starts with them, so they must win the early DMA bandwidth
    xts = []
    xpool = ctx.enter_context(tc.tile_pool(name="xin", bufs=1))
    for t in range(LC):
        xt = xpool.tile([P, E], BF16, tag=f"x{t}")
        sync.dma_start(xt[:], x_t[t])
        xts.append(xt)
    ident = consts.tile([P, P], BF16)
    sync.dma_start(ident[:], id_d.ap())
    eps_ln = consts.tile([P, 1], F32)
    nc.vector.memset(eps_ln[:], LN_EPS)
    onescol = consts.tile([1, P], BF16)
    nc.vector.memset(onescol[:], 1.0)
    onesrow = consts.tile([1, 512], BF16)
    nc.vector.memset(onesrow[:], 1.0)
    # selectors for the denominator broadcast: cols 0:128 select partitions
    # 0:64 (head A), cols 128:256 select partitions 64:128 (head B); each is
    # used as a rank-1 matmul lhsT to replicate a row across its partitions
    sel2 = consts.tile([1, 2 * P], BF16)
    sync.dma_start(sel2[:], sel_d.ap())
    bvr = consts.tile([1, FL], BF16)
    sync.dma_start(bvr[:], bv_d.ap())
    bqr = consts.tile([1, FL], BF16)
    sync.dma_start(bqr[:], bq_d.ap())
    bkr = consts.tile([1, FL], BF16)
    sync.dma_start(bkr[:], bk_d.ap())
    wo_sb = consts.tile([P, FC, E], BF16)

    xhatT = consts.tile([P, EC, L], BF16)   # xhat transposed: [e, l]
    qT = consts.tile([P, FC, L], BF16)      # Q^T: [f, l] (scale folded in)
    kT = consts.tile([P, FC, L], BF16)      # K^T: [f, l]
    vaug = consts.tile([P, LC, HPC, 65], BF16)  # V | ones column, per l-chunk/head
    otun = consts.tile([P, FC, L], BF16)    # unnormalized attention output^T
    otall = consts.tile([P, FC, L], BF16)   # normalized attention output^T
    nc.vector.memset(vaug[:, :, :, 64:65], 1.0)

    # ---- Phases A+B interleaved: LayerNorm + PE transposes + projections ----
    # Emission order matters: the PE stream is in-order, so projections over
    # the first half of the sequence are emitted right after LN tiles 0-3,
    # keeping PE dense (and HAM warm) while LN tiles 4-7 still stream.
    with tc.tile_pool(name="stats", bufs=6) as statp, \
         tc.tile_pool(name="xh", bufs=3) as xhp, \
         tc.tile_pool(name="w", bufs=1) as wpool, \
         tc.tile_pool(name="tp", bufs=2, space="PSUM") as tpp, \
         tc.tile_pool(name="pjqk", bufs=4, space="PSUM") as pjqk, \
         tc.tile_pool(name="pjv", bufs=2, space="PSUM") as pjv:
        wq_sb = wpool.tile([P, EC, FL], BF16)
        sync.dma_start(wq_sb[:], wq_d.ap())
        wk_sb = wpool.tile([P, EC, FL], BF16)
        sync.dma_start(wk_sb[:], wk_d.ap())
        wv_sb = wpool.tile([P, EC, FL], BF16)
        sync.dma_start(wv_sb[:], wv_d.ap())

        def ln_tile(t):
            xt = xts[t]
            st = statp.tile([P, 2, 6], F32)
            nc.vector.bn_stats(st[:, 0, :], xt[:, 0:512])
            nc.vector.bn_stats(st[:, 1, :], xt[:, 512:1024])
            mv = statp.tile([P, 2], F32)
            nc.vector.bn_aggr(mv[:], st[:])
            srt = statp.tile([P, 1], F32)
            nc.scalar.activation(srt[:], mv[:, 1:2], AF.Sqrt, bias=eps_ln[:],
                                 scale=1.0)
            rstd = statp.tile([P, 1], F32)
            nc.vector.reciprocal(rstd[:], srt[:])
            xh = xhp.tile([P, E], BF16)
            nc.vector.tensor_scalar(xh[:], xt[:], mv[:, 0:1], rstd[:],
                                    op0=ALU.subtract, op1=ALU.mult)
            # transpose [128,128] blocks on the (otherwise idle) PE, packing
            # four per PSUM bank so one wide DVE copy drains each group
            for g in range(2):
                tp = tpp.tile([P, 4 * P], BF16, tag="tp")
                for j in range(4):
                    et = 4 * g + j
                    nc.tensor.transpose(tp[:, j * P:(j + 1) * P],
                                        xh[:, bass.ts(et, P)], ident[:])
                nc.vector.tensor_copy(
                    xhatT[:, 4 * g:4 * g + 4, bass.ts(t, P)],
                    tp[:].rearrange("p (j c) -> p j c", j=4))

        def proj_half(half):
            for fc in range(FC):
                for i, (w_sb, dest, brow) in enumerate(
                        ((wq_sb, qT, bqr), (wk_sb, kT, bkr))):
                    ps = pjqk.tile([P, 512], F32)
                    # bias via rank-1 accumulation: ps[f, l] += b[f] * 1
                    nc.tensor.matmul(ps[:],
                                     lhsT=brow[0:1, fc * P:(fc + 1) * P],
                                     rhs=onesrow[:],
                                     start=True, stop=False)
                    for ec in range(EC):
                        nc.tensor.matmul(
                            ps[:],
                            lhsT=w_sb[:, ec, fc * P:(fc + 1) * P],
                            rhs=xhatT[:, ec, half * 512:(half + 1) * 512],
                            start=False, stop=(ec == EC - 1))
                    if (fc + i) % 2 == 0:
                        nc.scalar.copy(
                            dest[:, fc, half * 512:(half + 1) * 512], ps[:])
                    else:
                        nc.vector.tensor_copy(
                            dest[:, fc, half * 512:(half + 1) * 512], ps[:])
            for lc in range(half * 4, half * 4 + 4):
                ps = pjv.tile([P, FL], F32)
                nc.tensor.matmul(ps[:], lhsT=onescol[:], rhs=bvr[:],
                                 start=True, stop=False)
                for ec in range(EC):
                    nc.tensor.matmul(ps[:], lhsT=xhatT[:, ec, bass.ts(lc, P)],
                                     rhs=wv_sb[:, ec, :],
                                     start=False, stop=(ec == EC - 1))
                nc.vector.tensor_copy(vaug[:, lc, :, 0:64],
                                      ps[:].rearrange("p (h d) -> p h d", h=HPC))

        for t in range(4):
            ln_tile(t)
        proj_half(0)
        for t in range(4, LC):
            ln_tile(t)
        proj_half(1)

    # out-projection weights aren't needed until the very end; load them
    # once the front-critical DMAs have been issued
    sync.dma_start(wo_sb[:], wo_d.ap())

    # ---- Phase C: attention, 3-stage head pipeline ----
    with tc.tile_pool(name="el", bufs=2) as elp, \
         tc.tile_pool(name="rows", bufs=2) as rowp, \
         tc.tile_pool(name="lg", bufs=2, space="PSUM") as lg, \
         tc.tile_pool(name="otp", bufs=2, space="PSUM") as otp:
        ats = {}    # (h, kc) -> egb tile holding at = exp(logits)*egb
        ots = {}    # h -> [65, L] PSUM accumulator (row 64 = denominator)
        denbs = {}  # h -> [1, L] bf16 reciprocal denominator

        def qk_exp_mul(h, kc):
            fc, po = h // 2, (h % 2) * 64
            egbt = egbp.tile([P, L], BF16, tag="egb")
            sync.dma_start(egbt[:], eg_d.ap()[h, kc])
            lgt = lg.tile([P, L], F32, tag="lg")
            for half in range(2):
                nc.tensor.matmul(
                    lgt[:, half * 512:(half + 1) * 512],
                    lhsT=kT[po:po + 64, fc, bass.ts(kc, P)],
                    rhs=qT[po:po + 64, fc, half * 512:(half + 1) * 512],
                    start=True, stop=True)
            el = elp.tile([P, L], BF16, tag="el")
            nc.scalar.activation(el[:], lgt[:], AF.Exp)
            # at = el * egb, written in place over the egb tile
            nc.vector.tensor_mul(egbt[:], el[:], egbt[:])
            ats[(h, kc)] = egbt

        def av(h, kc):
            ot_ps = ots[h]
            at = ats.pop((h, kc))
            for half in range(2):
                nc.tensor.matmul(
                    ot_ps[:, half * 512:(half + 1) * 512],
                    lhsT=vaug[:, kc, h, :],
                    rhs=at[:, half * 512:(half + 1) * 512],
                    start=(kc == 0), stop=(kc == LC - 1))

        def norm(h):
            fc, po = h // 2, (h % 2) * 64
            ot_ps = ots.pop(h)
            nc.vector.tensor_copy(otun[po:po + 64, fc, :], ot_ps[0:64, :])
            s0 = rowp.tile([1, L], F32, tag="s0")
            if h % 2 == 0:
                nc.scalar.copy(s0[:], ot_ps[64:65, :])
            else:
                nc.vector.tensor_copy(s0[:], ot_ps[64:65, :])
            rr = rowp.tile([1, L], F32, tag="rr")
            nc.vector.reciprocal_approx_fast(rr[:], s0[:])
            rb = rowp.tile([1, L], BF16, tag=f"rb{h % 2}")
            nc.vector.tensor_copy(rb[:], rr[:])
            denbs[h] = rb
            if h % 2 == 1:
                qsb = lg.tile([P, L], F32, tag="lg")
                for half in range(2):
                    nc.tensor.matmul(
                        qsb[:, half * 512:(half + 1) * 512],
                        lhsT=sel2[0:1, 0:P],
                        rhs=denbs[h - 1][0:1, half * 512:(half + 1) * 512],
                        start=True, stop=False)
                    nc.tensor.matmul(
                        qsb[:, half * 512:(half + 1) * 512],
                        lhsT=sel2[0:1, P:2 * P],
                        rhs=rb[0:1, half * 512:(half + 1) * 512],
                        start=False, stop=True)
                nc.vector.tensor_mul(otall[:, fc, :], otun[:, fc, :], qsb[:])
                del denbs[h - 1]

        for ph in range(HPC + 2):
            if ph >= 2:
                norm(ph - 2)
            if 1 <= ph <= HPC:
                ot_new = otp.tile([65, L], F32, tag="ot")
                ots[ph - 1] = ot_new
            # emit the previous head's 16 AV matmuls as one contiguous
            # accumulation block (long same-bank chains keep HAM at 2.4 GHz),
            # then this head's QK/exp/mul ticks
            if 1 <= ph <= HPC:
                for kc in range(LC):
                    av(ph - 1, kc)
            if ph < HPC:
                for kc in range(LC):
                    qk_exp_mul(ph, kc)

    # ---- Phase D: output projection (partial, host masks/pairs/adds rest) ----
    with tc.tile_pool(name="op", bufs=2, space="PSUM") as op, \
         tc.tile_pool(name="outs", bufs=3) as outp:
        for lc in range(LC):
            ps = op.tile([P, E], F32)
            for half in range(2):
                for fc in range(FC):
                    nc.tensor.matmul(
                        ps[:, half * 512:(half + 1) * 512],
                        lhsT=otall[:, fc, bass.ts(lc, P)],
                        rhs=wo_sb[:, fc, half * 512:(half + 1) * 512],
                        start=(fc == 0), stop=(fc == FC - 1))
            ot = outp.tile([P, E], BF16)
            nc.scalar.copy(ot[:, 0:512], ps[:, 0:512])
            nc.vector.tensor_copy(ot[:, 512:1024], ps[:, 512:1024])
            sync.dma_start(out_t[lc], ot[:])


def build_nc():
    nc = bacc.Bacc("TRN2", target_bir_lowering=False, debug=False)
    xd = nc.dram_tensor("x", [L, E], BF16, kind="ExternalInput")
    wq_d = nc.dram_tensor("wqT", [P, EC, FL], BF16, kind="ExternalInput")
    wk_d = nc.dram_tensor("wkT", [P, EC, FL], BF16, kind="ExternalInput")
    wv_d = nc.dram_tensor("wvT", [P, EC, FL], BF16, kind="ExternalInput")
    wo_d = nc.dram_tensor("woT", [P, FC, E], BF16, kind="ExternalInput")
    bq_d = nc.dram_tensor("bqr", [1, FL], BF16, kind="ExternalInput")
    bk_d = nc.dram_tensor("bkr", [1, FL], BF16, kind="ExternalInput")
    bv_d = nc.dram_tensor("bvr", [1, FL], BF16, kind="ExternalInput")
    eg_d = nc.dram_tensor("egb", [HPC, LC, P, L], BF16, kind="ExternalInput")
    id_d = nc.dram_tensor("ident", [P, P], BF16, kind="ExternalInput")
    sel_d = nc.dram_tensor("sel2", [1, 2 * P], BF16, kind="ExternalInput")
    out_d = nc.dram_tensor("partial", [L, E], BF16, kind="ExternalOutput")
    with tile.TileContext(nc) as tc, ExitStack() as ctx:
        _emit(nc, tc, ctx, xd, wq_d, wk_d, wv_d, wo_d, bq_d, bk_d, bv_d, eg_d,
              id_d, sel_d, out_d)
    nc.compile()
    return nc


def _wdev(w):
    # [FL, E] slice of an LN-folded weight -> lhsT layout [P, EC, FL]
    return np.ascontiguousarray(
        w.T.reshape(EC, P, FL).transpose(1, 0, 2)).astype(NBF16)


def prepare_in_maps(x, bias, mask, Wq, bq, Wk, bk, Wv, bv, Wo, bo, gamma, beta, gate):
    x = np.asarray(x, np.float32)
    gamma = np.asarray(gamma, np.float32)
    beta = np.asarray(beta, np.float32)
    gate = np.asarray(gate, np.float32)
    Wq = np.asarray(Wq, np.float32)
    Wk = np.asarray(Wk, np.float32)
    Wv = np.asarray(Wv, np.float32)
    Wo = np.asarray(Wo, np.float32)
    bq = np.asarray(bq, np.float32)
    bk = np.asarray(bk, np.float32)
    bv = np.asarray(bv, np.float32)
    scale = 1.0 / np.sqrt(np.float32(D))

    Wqe = (Wq * gamma[None, :]) * scale
    Wke = Wk * gamma[None, :]
    Wve = Wv * gamma[None, :]
    bqe = (bq + Wq @ beta) * scale
    bke = bk + Wk @ beta
    bve = bv + Wv @ beta
    mf = np.asarray(mask, np.float32)

    in_maps = []
    for c in range(NCORES):
        b, h0 = c // 2, (c % 2) * HPC
        sl = slice(h0 * D, h0 * D + FL)
        g = gate[h0:h0 + HPC]
        bb = np.asarray(bias[b, h0:h0 + HPC], np.float32)      # [HPC, q, k]
        egb = np.exp(g[:, None, None] * bb)
        egb *= mf[b][None, None, :]                            # key mask
        # floor keeps every denominator nonzero (no NaN guard on device);
        # the query mask is applied host-side in finish()
        np.maximum(egb, 1e-30, out=egb)
        egbT = np.ascontiguousarray(egb.transpose(0, 2, 1))    # [HPC, k, q]
        in_maps.append({
            "x": np.ascontiguousarray(x[b]).astype(NBF16),
            "wqT": _wdev(Wqe[sl]),
            "wkT": _wdev(Wke[sl]),
            "wvT": _wdev(Wve[sl]),
            "woT": np.ascontiguousarray(
                Wo[:, sl].T.reshape(FC, P, E).transpose(1, 0, 2)).astype(NBF16),
            "bqr": bqe[sl].reshape(1, FL).astype(NBF16),
            "bkr": bke[sl].reshape(1, FL).astype(NBF16),
            "bvr": bve[sl].reshape(1, FL).astype(NBF16),
            "egb": egbT.reshape(HPC, LC, P, L).astype(NBF16),
            "ident": np.eye(P, dtype=NBF16),
            "sel2": np.kron(np.eye(2), np.ones((1, 64))).reshape(1, 2 * P).astype(NBF16),
        })
    return in_maps


def finish(x, mask, bo, partials):
    x = np.asarray(x, np.float32)
    bo = np.asarray(bo, np.float32)
    mf = np.asarray(mask, np.float32)
    out = np.empty((B, L, E), np.float32)
    for b in range(B):
        p = partials[2 * b].astype(np.float32) + partials[2 * b + 1].astype(np.float32)
        out[b] = x[b] + mf[b][:, None] * p + bo[None, :]
    return out


def run_spmd(in_maps, trace=False, trace_cores=None, **kw):
    global _NC
    if _NC is None:
        _NC = build_nc()
    return run_bass_kernel_spmd(_NC, in_maps, core_ids=list(range(NCORES)),
                                trace=trace, trace_cores=trace_cores, **kw)


def kernel(**inputs):
    in_maps = prepare_in_maps(**inputs)
    res = run_spmd(in_maps)
    partials = [r["partial"] for r in res.results]
    return finish(inputs["x"], inputs["mask"], inputs["bo"], partials)


# revision 39
# speedup vs baseline: 1.6765x; 1.6765x over previous
"""Trainium2 Bass kernel for nn_BiasedMultiHeadAttention (B=4, H=16, L=1024, E=1024).

Masked-position compaction: queries and keys share mask[b], and masked
positions contribute nothing to the output (masked keys have zero attention
weight; masked query rows are zeroed). The host permutes each batch so the
~M_b unmasked positions come first, pads to M (multiple of 128), and the
device runs the whole block on [M, E] instead of [L, E]: projections/LN scale
by M/L and the attention quadratics (logits, exp, AV, egb traffic) by (M/L)^2.
The host scatters the [M, E] partial back into [L, E] rows.

Sharding: 64 (batch, head) pairs over 8 cores -> core c handles batch b=c//2,
heads h0=(c%2)*8 .. h0+8. Per core: LayerNorm + Q/K/V projection slices +
biased attention for its 8 heads + its slice of the output projection
(row-parallel); the two cores sharing a batch each return a partial [M, E]
out-projection (bf16); the host sums the pair, applies the query mask, and
adds residual + bo.

Host-side folding (exact algebra, fp32): gamma/beta into the projection
weights/biases; 1/sqrt(D) into Wq/bq; gate*bias pre-exponentiated with the
key mask and a 1e-30 floor (keeps denominators nonzero -> no NaN guard).

Device: attention runs transposed, logitsT[k, q]; the softmax denominator
falls out of the AV matmul via an appended ones-column on V. 3-stage head
pipeline: QK/exp/egb-mul for head h streams while head h-1's AV matmuls run
as one contiguous accumulation block from fully staged tiles (keeps HAM at
2.4 GHz), and head h-2's normalization (reciprocal-denominator broadcast via
rank-1 matmuls) trails behind.
"""
import numpy as np
import ml_dtypes
from contextlib import ExitStack

import concourse.bass as bass
import concourse.bacc as bacc
import concourse.tile as tile
from concourse import mybir
from concourse.bass_utils import run_bass_kernel_spmd

BF16 = mybir.dt.bfloat16
F32 = mybir.dt.float32
NBF16 = ml_dtypes.bfloat16
AF = mybir.ActivationFunctionType
ALU = mybir.AluOpType

P = 128
B, L, E, D, H = 4, 1024, 1024, 64, 16
HPC = 8            # heads per core
FL = HPC * D       # local feature width = 512
FC = FL // P       # 4 feature chunks
EC = E // P        # 8 embed chunks
NCORES = 8
LN_EPS = 1e-5

_NCS = {}


def _nchunks(m):
    """Split a free span of m columns into PSUM-bank-sized (<=512) chunks."""
    out, o = [], 0
    while o < m:
        c = min(512, m - o)
        out.append((o, c))
        o += c
    return out


def _emit(nc, tc, ctx, M, xd, wq_d, wk_d, wv_d, wo_d, bq_d, bk_d, bv_d, eg_d,
          id_d, sel_d, out_d):
    MC = M // P
    sync = nc.sync
    x_t = xd.ap().rearrange("(t p) e -> t p e", p=P)
    out_t = out_d.ap().rearrange("(t p) e -> t p e", p=P)

    consts = ctx.enter_context(tc.tile_pool(name="consts", bufs=1))
    # egb prefetch pool lives at outer scope so its SBUF cannot alias the
    # projection-phase weight pool: egb streams during the projection phase
    # instead of stalling the attention loop.
    egbp = ctx.enter_context(tc.tile_pool(name="egb", bufs=20))

    ident = consts.tile([P, P], BF16)
    sync.dma_start(ident[:], id_d.ap())
    eps_ln = consts.tile([P, 1], F32)
    nc.vector.memset(eps_ln[:], LN_EPS)
    onescol = consts.tile([1, P], BF16)
    nc.vector.memset(onescol[:], 1.0)
    onesrow = consts.tile([1, 512], BF16)
    nc.vector.memset(onesrow[:], 1.0)
    # selectors for the denominator broadcast: cols 0:128 select partitions
    # 0:64 (head A), cols 128:256 select partitions 64:128 (head B)
    sel2 = consts.tile([1, 2 * P], BF16)
    sync.dma_start(sel2[:], sel_d.ap())
    bvr = consts.tile([1, FL], BF16)
    sync.dma_start(bvr[:], bv_d.ap())
    bqr = consts.tile([1, FL], BF16)
    sync.dma_start(bqr[:], bq_d.ap())
    bkr = consts.tile([1, FL], BF16)
    sync.dma_start(bkr[:], bk_d.ap())
    wo_sb = consts.tile([P, FC, E], BF16)

    xhatT = consts.tile([P, EC, M], BF16)   # xhat transposed: [e, m]
    qT = consts.tile([P, FC, M], BF16)      # Q^T: [f, m] (scale folded in)
    kT = consts.tile([P, FC, M], BF16)      # K^T: [f, m]
    vaug = consts.tile([P, MC, HPC, 65], BF16)  # V | ones column, per m-chunk/head
    otun = consts.tile([P, FC, M], BF16)    # unnormalized attention output^T
    otall = consts.tile([P, FC, M], BF16)   # normalized attention output^T
    nc.vector.memset(vaug[:, :, :, 64:65], 1.0)

    # ---- Phases A+B interleaved: LayerNorm + PE transposes + projections ----
    with tc.tile_pool(name="xin", bufs=1) as xpool, \
         tc.tile_pool(name="stats", bufs=6) as statp, \
         tc.tile_pool(name="xh", bufs=3) as xhp, \
         tc.tile_pool(name="w", bufs=1) as wpool, \
         tc.tile_pool(name="tp", bufs=2, space="PSUM") as tpp, \
         tc.tile_pool(name="pjqk", bufs=4, space="PSUM") as pjqk, \
         tc.tile_pool(name="pjv", bufs=2, space="PSUM") as pjv:
        # x tiles first: the LayerNorm -> transpose -> projection critical
        # path starts with them, so they must win the early DMA bandwidth
        xts = []
        for t in range(MC):
            xt = xpool.tile([P, E], BF16, tag=f"x{t}")
            sync.dma_start(xt[:], x_t[t])
            xts.append(xt)
        wq_sb = wpool.tile([P, EC, FL], BF16)
        sync.dma_start(wq_sb[:], wq_d.ap())
        wk_sb = wpool.tile([P, EC, FL], BF16)
        sync.dma_start(wk_sb[:], wk_d.ap())
        wv_sb = wpool.tile([P, EC, FL], BF16)
        sync.dma_start(wv_sb[:], wv_d.ap())

        def ln_tile(t):
            xt = xts[t]
            st = statp.tile([P, 2, 6], F32)
            nc.vector.bn_stats(st[:, 0, :], xt[:, 0:512])
            nc.vector.bn_stats(st[:, 1, :], xt[:, 512:1024])
            mv = statp.tile([P, 2], F32)
            nc.vector.bn_aggr(mv[:], st[:])
            srt = statp.tile([P, 1], F32)
            nc.scalar.activation(srt[:], mv[:, 1:2], AF.Sqrt, bias=eps_ln[:],
                                 scale=1.0)
            rstd = statp.tile([P, 1], F32)
            nc.vector.reciprocal(rstd[:], srt[:])
            xh = xhp.tile([P, E], BF16)
            nc.vector.tensor_scalar(xh[:], xt[:], mv[:, 0:1], rstd[:],
                                    op0=ALU.subtract, op1=ALU.mult)
            # transpose [128,128] blocks on the (otherwise idle) PE, packing
            # four per PSUM bank so one wide DVE copy drains each group
            for g in range(2):
                tp = tpp.tile([P, 4 * P], BF16, tag="tp")
                for j in range(4):
                    et = 4 * g + j
                    nc.tensor.transpose(tp[:, j * P:(j + 1) * P],
                                        xh[:, bass.ts(et, P)], ident[:])
                nc.vector.tensor_copy(
                    xhatT[:, 4 * g:4 * g + 4, bass.ts(t, P)],
                    tp[:].rearrange("p (j c) -> p j c", j=4))

        def qk_cols(o, csz):
            # q/k projections over xhatT columns [o, o+csz)
            for fc in range(FC):
                for i, (w_sb, dest, brow) in enumerate(
                        ((wq_sb, qT, bqr), (wk_sb, kT, bkr))):
                    ps = pjqk.tile([P, 512], F32, tag="pjqk")
                    # bias via rank-1 accumulation: ps[f, m] += b[f] * 1
                    nc.tensor.matmul(ps[:, 0:csz],
                                     lhsT=brow[0:1, fc * P:(fc + 1) * P],
                                     rhs=onesrow[0:1, 0:csz],
                                     start=True, stop=False)
                    for ec in range(EC):
                        nc.tensor.matmul(
                            ps[:, 0:csz],
                            lhsT=w_sb[:, ec, fc * P:(fc + 1) * P],
                            rhs=xhatT[:, ec, o:o + csz],
                            start=False, stop=(ec == EC - 1))
                    if (fc + i) % 2 == 0:
                        nc.scalar.copy(dest[:, fc, o:o + csz], ps[:, 0:csz])
                    else:
                        nc.vector.tensor_copy(dest[:, fc, o:o + csz],
                                              ps[:, 0:csz])

        def v_tiles(lcs):
            for lc in lcs:
                ps = pjv.tile([P, FL], F32)
                nc.tensor.matmul(ps[:], lhsT=onescol[:], rhs=bvr[:],
                                 start=True, stop=False)
                for ec in range(EC):
                    nc.tensor.matmul(ps[:], lhsT=xhatT[:, ec, bass.ts(lc, P)],
                                     rhs=wv_sb[:, ec, :],
                                     start=False, stop=(ec == EC - 1))
                nc.vector.tensor_copy(vaug[:, lc, :, 0:64],
                                      ps[:].rearrange("p (h d) -> p h d", h=HPC))

        nt0 = min(4, MC)
        for t in range(nt0):
            ln_tile(t)
        chunks = _nchunks(M)
        qk_cols(chunks[0][0], chunks[0][1])
        v_tiles(range(nt0))
        for t in range(nt0, MC):
            ln_tile(t)
        for o, csz in chunks[1:]:
            qk_cols(o, csz)
        v_tiles(range(nt0, MC))

    # out-projection weights aren't needed until the very end; load them
    # once the front-critical DMAs have been issued
    sync.dma_start(wo_sb[:], wo_d.ap())

    # ---- Phase C: attention, 3-stage head pipeline ----
    with tc.tile_pool(name="el", bufs=2) as elp, \
         tc.tile_pool(name="rows", bufs=2) as rowp, \
         tc.tile_pool(name="lg", bufs=2, space="PSUM") as lg, \
         tc.tile_pool(name="otp", bufs=2, space="PSUM") as otp:
        ats = {}    # (h, kc) -> egb tile holding at = exp(logits)*egb
        ots = {}    # h -> [65, M] PSUM accumulator (row 64 = denominator)
        denbs = {}  # h -> [1, M] bf16 reciprocal denominator

        def qk_exp_mul(h, kc):
            fc, po = h // 2, (h % 2) * 64
            egbt = egbp.tile([P, M], BF16, tag="egb")
            sync.dma_start(egbt[:], eg_d.ap()[h, kc])
            lgt = lg.tile([P, M], F32, tag="lg")
            for o, csz in _nchunks(M):
                nc.tensor.matmul(
                    lgt[:, o:o + csz],
                    lhsT=kT[po:po + 64, fc, bass.ts(kc, P)],
                    rhs=qT[po:po + 64, fc, o:o + csz],
                    start=True, stop=True)
            el = elp.tile([P, M], BF16, tag="el")
            nc.scalar.activation(el[:], lgt[:], AF.Exp)
            # at = el * egb, written in place over the egb tile
            nc.vector.tensor_mul(egbt[:], el[:], egbt[:])
            ats[(h, kc)] = egbt

        def av(h, kc):
            ot_ps = ots[h]
            at = ats.pop((h, kc))
            for o, csz in _nchunks(M):
                nc.tensor.matmul(
                    ot_ps[:, o:o + csz],
                    lhsT=vaug[:, kc, h, :],
                    rhs=at[:, o:o + csz],
                    start=(kc == 0), stop=(kc == MC - 1))

        def norm(h):
            fc, po = h // 2, (h % 2) * 64
            ot_ps = ots.pop(h)
            nc.vector.tensor_copy(otun[po:po + 64, fc, :], ot_ps[0:64, :])
            s0 = rowp.tile([1, M], F32, tag="s0")
            if h % 2 == 0:
                nc.scalar.copy(s0[:], ot_ps[64:65, :])
            else:
                nc.vector.tensor_copy(s0[:], ot_ps[64:65, :])
            rr = rowp.tile([1, M], F32, tag="rr")
            nc.vector.reciprocal_approx_fast(rr[:], s0[:])
            rb = rowp.tile([1, M], BF16, tag=f"rb{h % 2}")
            nc.vector.tensor_copy(rb[:], rr[:])
            denbs[h] = rb
            if h % 2 == 1:
                qsb = lg.tile([P, M], F32, tag="lg")
                for o, csz in _nchunks(M):
                    nc.tensor.matmul(
                        qsb[:, o:o + csz],
                        lhsT=sel2[0:1, 0:P],
                        rhs=denbs[h - 1][0:1, o:o + csz],
                        start=True, stop=False)
                    nc.tensor.matmul(
                        qsb[:, o:o + csz],
                        lhsT=sel2[0:1, P:2 * P],
                        rhs=rb[0:1, o:o + csz],
                        start=False, stop=True)
                nc.vector.tensor_mul(otall[:, fc, :], otun[:, fc, :], qsb[:])
                del denbs[h - 1]

        for ph in range(HPC + 2):
            if ph >= 2:
                norm(ph - 2)
            if 1 <= ph <= HPC:
                ot_new = otp.tile([65, M], F32, tag="ot")
                ots[ph - 1] = ot_new
                # previous head's AV matmuls as one contiguous accumulation
                # block (long same-bank chains keep HAM at 2.4 GHz)
                for kc in range(MC):
                    av(ph - 1, kc)
            if ph < HPC:
                for kc in range(MC):
                    qk_exp_mul(ph, kc)

    # ---- Phase D: output projection (partial, host masks/pairs/adds rest) ----
    with tc.tile_pool(name="op", bufs=2, space="PSUM") as op, \
         tc.tile_pool(name="outs", bufs=3) as outp:
        for lc in range(MC):
            ps = op.tile([P, E], F32)
            for half in range(2):
                for fc in range(FC):
                    nc.tensor.matmul(
                        ps[:, half * 512:(half + 1) * 512],
                        lhsT=otall[:, fc, bass.ts(lc, P)],
                        rhs=wo_sb[:, fc, half * 512:(half + 1) * 512],
                        start=(fc == 0), stop=(fc == FC - 1))
            ot = outp.tile([P, E], BF16)
            nc.scalar.copy(ot[:, 0:512], ps[:, 0:512])
            nc.vector.tensor_copy(ot[:, 512:1024], ps[:, 512:1024])
            sync.dma_start(out_t[lc], ot[:])


def build_nc(M):
    nc = bacc.Bacc("TRN2", target_bir_lowering=False, debug=False)
    MC = M // P
    xd = nc.dram_tensor("x", [M, E], BF16, kind="ExternalInput")
    wq_d = nc.dram_tensor("wqT", [P, EC, FL], BF16, kind="ExternalInput")
    wk_d = nc.dram_tensor("wkT", [P, EC, FL], BF16, kind="ExternalInput")
    wv_d = nc.dram_tensor("wvT", [P, EC, FL], BF16, kind="ExternalInput")
    wo_d = nc.dram_tensor("woT", [P, FC, E], BF16, kind="ExternalInput")
    bq_d = nc.dram_tensor("bqr", [1, FL], BF16, kind="ExternalInput")
    bk_d = nc.dram_tensor("bkr", [1, FL], BF16, kind="ExternalInput")
    bv_d = nc.dram_tensor("bvr", [1, FL], BF16, kind="ExternalInput")
    eg_d = nc.dram_tensor("egb", [HPC, MC, P, M], BF16, kind="ExternalInput")
    id_d = nc.dram_tensor("ident", [P, P], BF16, kind="ExternalInput")
    sel_d = nc.dram_tensor("sel2", [1, 2 * P], BF16, kind="ExternalInput")
    out_d = nc.dram_tensor("partial", [M, E], BF16, kind="ExternalOutput")
    with tile.TileContext(nc) as tc, ExitStack() as ctx:
        _emit(nc, tc, ctx, M, xd, wq_d, wk_d, wv_d, wo_d, bq_d, bk_d, bv_d,
              eg_d, id_d, sel_d, out_d)
    nc.compile()
    return nc


def _wdev(w):
    # [FL, E] slice of an LN-folded weight -> lhsT layout [P, EC, FL]
    return np.ascontiguousarray(
        w.T.reshape(EC, P, FL).transpose(1, 0, 2)).astype(NBF16)


def _pick_m(mask):
    counts = np.asarray(mask).sum(axis=1)
    return max(P, int(np.ceil(counts.max() / P) * P))


def prepare_in_maps(x, bias, mask, Wq, bq, Wk, bk, Wv, bv, Wo, bo, gamma, beta,
                    gate, M=None):
    x = np.asarray(x, np.float32)
    gamma = np.asarray(gamma, np.float32)
    beta = np.asarray(beta, np.float32)
    gate = np.asarray(gate, np.float32)
    Wq = np.asarray(Wq, np.float32)
    Wk = np.asarray(Wk, np.float32)
    Wv = np.asarray(Wv, np.float32)
    Wo = np.asarray(Wo, np.float32)
    bq = np.asarray(bq, np.float32)
    bk = np.asarray(bk, np.float32)
    bv = np.asarray(bv, np.float32)
    scale = 1.0 / np.sqrt(np.float32(D))
    mf = np.asarray(mask, np.float32)
    if M is None:
        M = _pick_m(mask)
    MC = M // P

    Wqe = (Wq * gamma[None, :]) * scale
    Wke = Wk * gamma[None, :]
    Wve = Wv * gamma[None, :]
    bqe = (bq + Wq @ beta) * scale
    bke = bk + Wk @ beta
    bve = bv + Wv @ beta

    perms = [np.argsort(-mf[b], kind="stable")[:M] for b in range(B)]

    in_maps = []
    for c in range(NCORES):
        b, h0 = c // 2, (c % 2) * HPC
        idx = perms[b]
        sl = slice(h0 * D, h0 * D + FL)
        g = gate[h0:h0 + HPC]
        bb = np.asarray(bias[b, h0:h0 + HPC], np.float32)[:, idx][:, :, idx]
        egb = np.exp(g[:, None, None] * bb)
        egb *= mf[b][idx][None, None, :]                       # key mask
        # floor keeps every denominator nonzero (no NaN guard on device);
        # the query mask is applied host-side in finish()
        np.maximum(egb, 1e-30, out=egb)
        egbT = np.ascontiguousarray(egb.transpose(0, 2, 1))    # [HPC, k, q]
        in_maps.append({
            "x": np.ascontiguousarray(x[b][idx]).astype(NBF16),
            "wqT": _wdev(Wqe[sl]),
            "wkT": _wdev(Wke[sl]),
            "wvT": _wdev(Wve[sl]),
            "woT": np.ascontiguousarray(
                Wo[:, sl].T.reshape(FC, P, E).transpose(1, 0, 2)).astype(NBF16),
            "bqr": bqe[sl].reshape(1, FL).astype(NBF16),
            "bkr": bke[sl].reshape(1, FL).astype(NBF16),
            "bvr": bve[sl].reshape(1, FL).astype(NBF16),
            "egb": egbT.reshape(HPC, MC, P, M).astype(NBF16),
            "ident": np.eye(P, dtype=NBF16),
            "sel2": np.kron(np.eye(2), np.ones((1, 64))).reshape(1, 2 * P).astype(NBF16),
        })
    return in_maps, perms


def finish(x, mask, bo, partials, perms):
    x = np.asarray(x, np.float32)
    bo = np.asarray(bo, np.float32)
    mf = np.asarray(mask, np.float32)
    out = np.empty((B, L, E), np.float32)
    for b in range(B):
        idx = perms[b]
        p = (partials[2 * b].astype(np.float32)
             + partials[2 * b + 1].astype(np.float32))
        full = np.zeros((L, E), np.float32)
        full[idx] = p * mf[b][idx][:, None]
        out[b] = x[b] + full + bo[None, :]
    return out


def run_spmd(in_maps, M=None, trace=False, trace_cores=None, **kw):
    if M is None:
        M = in_maps[0]["egb"].shape[3]
    nc = _NCS.get(M)
    if nc is None:
        nc = _NCS[M] = build_nc(M)
    return run_bass_kernel_spmd(nc, in_maps, core_ids=list(range(NCORES)),
                                trace=trace, trace_cores=trace_cores, **kw)


def kernel(**inputs):
    M = _pick_m(inputs["mask"])
    in_maps, perms = prepare_in_maps(**inputs, M=M)
    res = run_spmd(in_maps, M)
    partials = [r["partial"] for r in res.results]
    return finish(inputs["x"], inputs["mask"], inputs["bo"], partials, perms)
